# revision 63
# baseline (speedup 1.0000x reference)
"""Trainium2 Bass kernel: float32 -> 32-channel bit-plane encoding.

For input x [4096, 512] f32, produces out [4096, 512, 32] f32 where
out[b, f, 0] = (x[b,f] < 0) and out[b, f, 1+j] = bit (30-j) of
bitcast_int32(|x[b,f]|), MSB first.

Host-side repack merges the sign test into bit 31:
  i' = (bitcast_u32(x) & 0x7FFFFFFF) | ((x < 0) << 31)
and splits i' into two u16 planes (hi = bits 31..16 -> channels 0..15,
lo = bits 15..0 -> channels 16..31), stored partition-major: SBUF row p
holds [rt0 hi|lo][rt1 hi|lo][rt2][rt3] for source rows rt*128+p, so the
rt1-3 bulk input loads as one 6KB-descriptor DMA.

Device compute is ONE fused DVE tensor_scalar per channel PAIR:
  t = (v >> s) & 0x0101        (u16 -> u16, both ops bitwise-class)
puts bit s in byte 0 and bit s+8 in byte 1 of the u16 lane — two final
u8 output channels per processed element.  With u16 in/out, packed,
SBUF-only operands this runs in the DVE 4x_2p perf mode (0.25
cyc/elem), so the whole 8.39M-byte/core output costs ~8.5us of VectorE
time; the kernel is out-DMA bound.  s in 0..7 over the hi plane covers
channel pairs (15-s, 7-s); over the lo plane (31-s, 23-s).

The device writes uint8 pairs (values exactly 0/1) laid out
[rows, 16 pair-planes, 512] u16, so each output row is one contiguous
16KB run in BOTH SBUF and DRAM: out-DMA descriptors stay large (the
~420 GB/s regime measured on this part).  Writing u8 instead of f32
cuts the out-DMA stream 4x (8.39 MB/core), moving the roofline from
~87us to ~23us.  The host reassembles [rows, 512, 32] channel order
with a fixed 32-wide permutation during the u8 -> f32 widening.

Scheduling (each point A/B-measured against the alternative on HW):
- rt0's hi plane loads as two parallel halves on the sync and scalar
  HWDGE queues; the scalar queue then brings rt0-lo and rt1.
- rt2-3 input rides the *slow* gpsimd SWDGE queue with no wait: its
  ~3us issue-to-land latency drops those bytes into the engine-idle
  hole between the in-stream drain and the first out piece (-0.6us).
- rt2 and rt3 are computed by 1024-elem instructions spanning both
  tiles (multi-dim AP, still 4x_2p), so their bytes are piece-ready
  ~4us earlier with no descriptor-size penalty (-0.4us).
- Out pieces are pair-plane ranges that grow geometrically (1,1,2,4,8
  planes of rt0, then rt1 whole), each waiting only on the VectorE
  instructions that filled it, and alternate between the sync and
  scalar queues so the next piece's descriptors are pre-generated
  while the previous drains (-0.5us).
- Because pair23 makes rt2+rt3 ready simultaneously, the whole 4.2MB
  tail ships as two parallel partition-half pieces (one per queue,
  single wait each).  DRAM row order [t, p, q, f] keeps descriptors at
  16KB — the measured sweet spot; the 32KB-descriptor variant
  ([p, t, q, f] order) traced at ~half the per-descriptor rate.
Measured ~25.5-26us/core: ~21-22us of saturated DMA (9.4MB at the
~420GB/s fabric ceiling; 16KB descriptors = 26.4B/ns/engine) + ~2us
in-DMA/compute ramp + ~1.5us fixed framework latencies.  Residual
engine-idle gaps late in the stream track all 8 cores peaking
together (HBM backpressure) and are not addressable per-core;
splitting the tail pieces finer loses more to 8KB-descriptor
efficiency than it recovers (measured).
"""

import sys

if "/opt/trn_rl_repo" not in sys.path:
    sys.path.insert(0, "/opt/trn_rl_repo")

import numpy as np

import concourse.bass as bass
import concourse.mybir as mybir

P = 128          # SBUF partitions
F = 512          # features per row
K = 32           # output channels per feature
NPAIR = 16       # channel-pair planes
N_CORES = 8
ROWS_TOTAL = 4096
ROWS = ROWS_TOTAL // N_CORES   # rows per core
NRT = ROWS // P                # row tiles per core (4)

# out-DMA pieces: (rt, q0, q1) — pair-plane ranges within a row tile.
# Early pieces are single planes so the out stream starts right after the
# first VectorE instruction; later pieces grow (sync issues one dma_start
# per piece at ~0.65us, so small pieces must stay ahead of the drain).
PIECES = [(0, 0, 1), (0, 1, 2), (0, 2, 4), (0, 4, 8), (0, 8, 16),
          (1, 0, 16),
          (2, 0, 16),
          (3, 0, 16)]

# plane j covers: j<8 -> hi plane, s=j, channels (15-j @byte0, 7-j @byte1)
#                 j>=8 -> lo plane, s=j-8, channels (31-s @byte0, 23-s @byte1)
_PLANE_K_SEQ = []
for _j in range(8):
    _PLANE_K_SEQ += [15 - _j, 7 - _j]
for _j in range(8):
    _PLANE_K_SEQ += [31 - _j, 23 - _j]
# PERM[k] = position of channel k in the device byte stream of one (row, f)
PERM = np.array([_PLANE_K_SEQ.index(k) for k in range(K)], dtype=np.int64)


def build_nc(pieces=None, tensor_q=True, gp_bulk=True,
             pair23=True, big23=True, b23_16k=True) -> bass.Bass:
    pieces = PIECES if pieces is None else pieces
    if big23:
        assert pair23  # rt2/rt3 must become ready together
        pieces = [pc for pc in pieces if pc[0] < 2] \
            + [("b23", 0, P // 2), ("b23", P // 2, P)]
    nc = bass.Bass("TRN2", target_bir_lowering=False, debug=False)
    u16 = mybir.dt.uint16
    SHR, AND = mybir.AluOpType.logical_shift_right, mybir.AluOpType.bitwise_and

    # xm is partition-major: row p holds [rt0 hi|lo][rt1 hi|lo][rt2][rt3]
    # for source row rt*128+p, so the rt1-3 bulk loads as one 6KB-descriptor
    # DMA (2KB row descriptors only reach ~20 B/ns; >=4KB reach ~26).
    xm = nc.declare_dram_parameter("xm", [P, NRT * 2 * F], u16,
                                   isOutput=False)
    out = nc.declare_dram_parameter("out", [ROWS * NPAIR, F], u16,
                                    isOutput=True)
    xm_ap, out_ap = xm.ap(), out.ap()
    # [r, q, f] view of out (q = pair plane, innermost block of each row)
    out_rqf = out_ap.rearrange("(r q) f -> r q f", q=NPAIR)
    if big23:
        # rows 256..511 reordered so each partition's rt2+rt3 bytes
        # (contiguous in poall) ship as one piece per partition-half:
        # [p, t, q, f] order gives 32KB descriptors, [t, p, q, f] keeps 16KB
        if b23_16k:
            out_b23 = out_ap[2 * P * NPAIR:, :] \
                .rearrange("(t p q) f -> p t q f", p=P, q=NPAIR)
        else:
            out_b23 = out_ap[2 * P * NPAIR:, :] \
                .rearrange("(p t q) f -> p t q f", t=2, q=NPAIR)

    from contextlib import ExitStack
    with ExitStack() as ctx:
        xtall = ctx.enter_context(
            nc.sbuf_tensor("xtall", [P, NRT * 2 * F], u16))

        def xsl(rt, a, b):
            return xtall[:, rt * 2 * F + a:rt * 2 * F + b]
        poall = ctx.enter_context(
            nc.sbuf_tensor("poall", [P, NRT * NPAIR * F], u16))
        po = [poall[:, rt * NPAIR * F:(rt + 1) * NPAIR * F]
              for rt in range(NRT)]

        in_sem = [ctx.enter_context(nc.semaphore(f"in_sem{b}"))
                  for b in range(2)]
        in0b_sem = ctx.enter_context(nc.semaphore("in0b_sem"))
        in23_sem = ctx.enter_context(nc.semaphore("in23_sem")) \
            if gp_bulk else None
        vd_sem = ctx.enter_context(nc.semaphore("vd_sem"))
        od_sem = ctx.enter_context(nc.semaphore("od_sem"))

        ctx.enter_context(nc.Block())
        block = nc.cur_block

        def emit_piece(eng, rt, q0, q1):
            if rt == "b23":        # p-half of the merged rt2+rt3 region
                p0, p1 = q0, q1
                eng.wait_ge(vd_sem, 3 * NPAIR)
                eng.dma_start(
                    out_b23[p0:p1, :, :, :],
                    poall[p0:p1, 2 * NPAIR * F:4 * NPAIR * F]
                    .rearrange("p (t q f) -> p t q f", t=2, q=NPAIR),
                ).then_inc(od_sem, 16)
                return
            # with pair23, one instruction covers plane q of BOTH rt2 and rt3
            base = 2 * NPAIR if (pair23 and rt >= 2) else rt * NPAIR
            eng.wait_ge(vd_sem, base + q1)
            eng.dma_start(
                out_rqf[rt * P:(rt + 1) * P, q0:q1, :],
                po[rt][:, q0 * F:q1 * F]
                .rearrange("p (q f) -> p q f", f=F),
            ).then_inc(od_sem, 16)

        @block.vector
        def _(vec: bass.BassEngine):
            for rt in range(2 if pair23 else NRT):
                for q in range(NPAIR):
                    if rt == 0 and q == 0:
                        vec.wait_ge(in_sem[0], 32)       # rt0 hi plane halves
                    elif rt == 0 and q == 8:
                        vec.wait_ge(in0b_sem, 32)        # rt0 lo plane halves
                    elif rt == 1 and q == 0:
                        vec.wait_ge(in_sem[1], 16)       # rt1(-3) bulk arrived
                    elif gp_bulk and rt == 2 and q == 0:
                        vec.wait_ge(in23_sem, 16)        # rt2-3 via gpsimd
                    plane, s = (0, q) if q < 8 else (F, q - 8)
                    o = po[rt][:, q * F:(q + 1) * F]
                    i0 = xsl(rt, plane, plane + F)
                    vec.tensor_scalar(o, i0, s, 0x0101, SHR, AND) \
                        .then_inc(vd_sem)
            if pair23:
                # one 1024-elem instruction per plane covering rt2 AND rt3:
                # both tiles' bytes become piece-ready ~4us earlier with no
                # descriptor-size penalty (still 4x_2p: last dim packed u16)
                o23 = poall[:, 2 * NPAIR * F:4 * NPAIR * F] \
                    .rearrange("p (r x) -> p r x", r=2)
                i23 = xtall[:, 4 * F:NRT * 2 * F] \
                    .rearrange("p (r x) -> p r x", r=2)
                for q in range(NPAIR):
                    if q == 0:
                        if gp_bulk:
                            vec.wait_ge(in23_sem, 16)
                    plane, s = (0, q) if q < 8 else (F, q - 8)
                    o = o23[:, :, q * F:(q + 1) * F]
                    i0 = i23[:, :, plane:plane + F]
                    vec.tensor_scalar(o, i0, s, 0x0101, SHR, AND) \
                        .then_inc(vd_sem)

        @block.scalar
        def _(sc: bass.BassEngine):
            # rt0 input split by PARTITION across the sync/scalar queues:
            # same two-queue parallelism as an f-split but with 1-2KB
            # descriptors instead of 512B (17.9 -> 21.6 B/ns)
            sc.dma_start(
                xtall[P // 2:P, 0:F], xm_ap[P // 2:P, 0:F]
            ).then_inc(in_sem[0], 16)
            sc.dma_start(
                xtall[P // 2:P, F:2 * F], xm_ap[P // 2:P, F:2 * F]
            ).then_inc(in0b_sem, 16)
            # rt1(-3) input bulk: large descriptors
            bulk_end = 4 * F if gp_bulk else NRT * 2 * F
            sc.dma_start(
                xtall[:, 2 * F:bulk_end], xm_ap[0:P, 2 * F:bulk_end]
            ).then_inc(in_sem[1], 16)
            if tensor_q:
                for i, (rt, q0, q1) in enumerate(pieces):
                    if i % 2 == 1:
                        emit_piece(sc, rt, q0, q1)

        if gp_bulk:
            # rt2-3 input rides the slow gpsimd SWDGE queue: its ~3us
            # issue-to-land latency drops these bytes into the engine-idle
            # hole between the in-stream drain and the first out piece.
            @block.gpsimd
            def _(gp: bass.BassEngine):
                gp.dma_start(
                    xtall[:, 4 * F:NRT * 2 * F],
                    xm_ap[0:P, 4 * F:NRT * 2 * F]
                ).then_inc(in23_sem, 16)

        @block.sync
        def _(sp: bass.BassEngine):
            # rt0 input: hi plane first so VectorE starts after 128KB
            sp.dma_start(
                xtall[0:P // 2, 0:F], xm_ap[0:P // 2, 0:F]
            ).then_inc(in_sem[0], 16)
            sp.dma_start(
                xtall[0:P // 2, F:2 * F], xm_ap[0:P // 2, F:2 * F]
            ).then_inc(in0b_sem, 16)
            for i, (rt, q0, q1) in enumerate(pieces):
                if tensor_q and i % 2 == 1:
                    continue
                emit_piece(sp, rt, q0, q1)

    return nc


_NC_CACHE = None


def _get_nc():
    global _NC_CACHE
    if _NC_CACHE is None:
        _NC_CACHE = build_nc()
    return _NC_CACHE


def pack_shard(x_shard: np.ndarray) -> np.ndarray:
    """[ROWS, F] f32 -> [P, NRT*2F] u16, partition-major: SBUF row p holds
    [rt0 hi|lo][rt1 hi|lo][rt2][rt3] for source rows rt*128+p, where hi is
    bits 31..16 (bit 31 replaced by the x<0 test) and lo is bits 15..0."""
    x_shard = np.ascontiguousarray(x_shard)
    xi = x_shard.view(np.uint32)
    xi = (xi & np.uint32(0x7FFFFFFF)) | \
        ((x_shard < 0).astype(np.uint32) << np.uint32(31))
    hi = (xi >> np.uint32(16)).astype(np.uint16)     # [ROWS, F]
    lo = (xi & np.uint32(0xFFFF)).astype(np.uint16)
    # [ROWS, 2, F] -> [NRT, P, 2, F] -> [P, NRT, 2, F] -> [P, NRT*2F]
    planes = np.stack([hi, lo], axis=1).reshape(NRT, P, 2, F)
    return np.ascontiguousarray(
        planes.transpose(1, 0, 2, 3).reshape(P, NRT * 2 * F))


def unpack_result(out_dev: np.ndarray, big23: bool = False,
                  b23_16k: bool = False) -> np.ndarray:
    """Device [ROWS*NPAIR, F] u16 -> [ROWS, F, K] f32 in channel order."""
    rqf = out_dev.reshape(ROWS, NPAIR, F)
    if big23 and not b23_16k:
        # rows 256..511 are stored partition-major [p, rt-2, q, f]
        head = rqf[:2 * P]
        tail = out_dev[2 * P * NPAIR:].reshape(P, 2, NPAIR, F) \
            .transpose(1, 0, 2, 3).reshape(2 * P, NPAIR, F)
        rqf = np.concatenate([head, tail], axis=0)
    # b23_16k keeps [t, p, q, f] = natural row order: no reshuffle needed
    raw = np.ascontiguousarray(rqf).view(np.uint8).reshape(ROWS, NPAIR, F, 2)
    byte_k = raw.transpose(0, 2, 1, 3).reshape(ROWS, F, K)
    return byte_k[:, :, PERM].astype(np.float32)


def kernel(x: np.ndarray) -> np.ndarray:
    from concourse.bass_utils import run_bass_kernel_spmd

    x = np.asarray(x, dtype=np.float32)
    assert x.shape == (ROWS_TOTAL, F), x.shape
    nc = _get_nc()
    in_maps = [
        {"xm": pack_shard(x[i * ROWS:(i + 1) * ROWS])} for i in range(N_CORES)
    ]
    res = run_bass_kernel_spmd(nc, in_maps, list(range(N_CORES)))
    parts = [unpack_result(res.results[i]["out"], big23=True, b23_16k=True)
             for i in range(N_CORES)]
    return np.concatenate(parts, axis=0)


# revision 64
# speedup vs baseline: 1.0108x; 1.0108x over previous
"""Trainium2 Bass kernel: float32 -> 32-channel bit-plane encoding.

For input x [4096, 512] f32, produces out [4096, 512, 32] f32 where
out[b, f, 0] = (x[b,f] < 0) and out[b, f, 1+j] = bit (30-j) of
bitcast_int32(|x[b,f]|), MSB first.

Host-side repack merges the sign test into bit 31:
  i' = (bitcast_u32(x) & 0x7FFFFFFF) | ((x < 0) << 31)
and splits i' into two u16 planes (hi = bits 31..16 -> channels 0..15,
lo = bits 15..0 -> channels 16..31), stored partition-major: SBUF row p
holds [rt0 hi|lo][rt1 hi|lo][rt2][rt3] for source rows rt*128+p, so the
rt1-3 bulk input loads as one 6KB-descriptor DMA.

Device compute is ONE fused DVE tensor_scalar per channel PAIR:
  t = (v >> s) & 0x0101        (u16 -> u16, both ops bitwise-class)
puts bit s in byte 0 and bit s+8 in byte 1 of the u16 lane — two final
u8 output channels per processed element.  With u16 in/out, packed,
SBUF-only operands this runs in the DVE 4x_2p perf mode (0.25
cyc/elem), so the whole 8.39M-byte/core output costs ~8.5us of VectorE
time; the kernel is out-DMA bound.  s in 0..7 over the hi plane covers
channel pairs (15-s, 7-s); over the lo plane (31-s, 23-s).

The device writes uint8 pairs (values exactly 0/1) laid out
[rows, 16 pair-planes, 512] u16, so each output row is one contiguous
16KB run in BOTH SBUF and DRAM: out-DMA descriptors stay large (the
~420 GB/s regime measured on this part).  Writing u8 instead of f32
cuts the out-DMA stream 4x (8.39 MB/core), moving the roofline from
~87us to ~23us.  The host reassembles [rows, 512, 32] channel order
with a fixed 32-wide permutation during the u8 -> f32 widening.

Scheduling (each point A/B-measured against the alternative on HW):
- rt0's hi plane loads as two parallel halves on the sync and scalar
  HWDGE queues; the scalar queue then brings rt0-lo and rt1.
- rt2-3 input rides the *slow* gpsimd SWDGE queue with no wait: its
  ~3us issue-to-land latency drops those bytes into the engine-idle
  hole between the in-stream drain and the first out piece (-0.6us).
- rt2 and rt3 are computed by 1024-elem instructions spanning both
  tiles (multi-dim AP, still 4x_2p), so their bytes are piece-ready
  ~4us earlier with no descriptor-size penalty (-0.4us).
- Out pieces are pair-plane ranges that grow geometrically (1,1,2,4,8
  planes of rt0, then rt1 whole), each waiting only on the VectorE
  instructions that filled it, and alternate between the sync and
  scalar queues so the next piece's descriptors are pre-generated
  while the previous drains (-0.5us).
- Because pair23 makes rt2+rt3 ready simultaneously, the whole 4.2MB
  tail ships as two parallel partition-half pieces (one per queue,
  single wait each).  DRAM row order [t, p, q, f] keeps descriptors at
  16KB — the measured sweet spot; the 32KB-descriptor variant
  ([p, t, q, f] order) traced at ~half the per-descriptor rate.
Measured ~25.5-26us/core: ~21-22us of saturated DMA (9.4MB at the
~420GB/s fabric ceiling; 16KB descriptors = 26.4B/ns/engine) + ~2us
in-DMA/compute ramp + ~1.5us fixed framework latencies.  Residual
engine-idle gaps late in the stream track all 8 cores peaking
together (HBM backpressure) and are not addressable per-core;
splitting the tail pieces finer loses more to 8KB-descriptor
efficiency than it recovers (measured).
"""

import sys

if "/opt/trn_rl_repo" not in sys.path:
    sys.path.insert(0, "/opt/trn_rl_repo")

import numpy as np

import concourse.bass as bass
import concourse.mybir as mybir

P = 128          # SBUF partitions
F = 512          # features per row
K = 32           # output channels per feature
NPAIR = 16       # channel-pair planes
N_CORES = 8
ROWS_TOTAL = 4096
ROWS = ROWS_TOTAL // N_CORES   # rows per core
NRT = ROWS // P                # row tiles per core (4)

# out-DMA pieces: (rt, q0, q1) — pair-plane ranges within a row tile.
# Early pieces are single planes so the out stream starts right after the
# first VectorE instruction; later pieces grow (sync issues one dma_start
# per piece at ~0.65us, so small pieces must stay ahead of the drain).
PIECES = [(0, 0, 1), (0, 1, 2), (0, 2, 4), (0, 4, 8), (0, 8, 16),
          (1, 0, 16),
          (2, 0, 16),
          (3, 0, 16)]

# plane j covers: j<8 -> hi plane, s=j, channels (15-j @byte0, 7-j @byte1)
#                 j>=8 -> lo plane, s=j-8, channels (31-s @byte0, 23-s @byte1)
_PLANE_K_SEQ = []
for _j in range(8):
    _PLANE_K_SEQ += [15 - _j, 7 - _j]
for _j in range(8):
    _PLANE_K_SEQ += [31 - _j, 23 - _j]
# PERM[k] = position of channel k in the device byte stream of one (row, f)
PERM = np.array([_PLANE_K_SEQ.index(k) for k in range(K)], dtype=np.int64)


def build_nc(pieces=None, tensor_q=True, gp_bulk=True,
             pair23=True, big23=True, b23_16k=True) -> bass.Bass:
    pieces = PIECES if pieces is None else pieces
    if big23:
        assert pair23  # rt2/rt3 must become ready together
        pieces = [pc for pc in pieces if pc[0] < 2] \
            + [("b23", 0, P // 2), ("b23", P // 2, P)]
    nc = bass.Bass("TRN2", target_bir_lowering=False, debug=False)
    u16 = mybir.dt.uint16
    SHR, AND = mybir.AluOpType.logical_shift_right, mybir.AluOpType.bitwise_and

    # xm is partition-major: row p holds [rt0 hi|lo][rt1 hi|lo][rt2][rt3]
    # for source row rt*128+p, so the rt1-3 bulk loads as one 6KB-descriptor
    # DMA (2KB row descriptors only reach ~20 B/ns; >=4KB reach ~26).
    xm = nc.declare_dram_parameter("xm", [P, NRT * 2 * F], u16,
                                   isOutput=False)
    out = nc.declare_dram_parameter("out", [ROWS * NPAIR, F], u16,
                                    isOutput=True)
    xm_ap, out_ap = xm.ap(), out.ap()
    # [r, q, f] view of out (q = pair plane, innermost block of each row)
    out_rqf = out_ap.rearrange("(r q) f -> r q f", q=NPAIR)
    if big23:
        # rows 256..511 reordered so each partition's rt2+rt3 bytes
        # (contiguous in poall) ship as one piece per partition-half:
        # [p, t, q, f] order gives 32KB descriptors, [t, p, q, f] keeps 16KB
        if b23_16k:
            out_b23 = out_ap[2 * P * NPAIR:, :] \
                .rearrange("(t p q) f -> p t q f", p=P, q=NPAIR)
        else:
            out_b23 = out_ap[2 * P * NPAIR:, :] \
                .rearrange("(p t q) f -> p t q f", t=2, q=NPAIR)

    from contextlib import ExitStack
    with ExitStack() as ctx:
        xtall = ctx.enter_context(
            nc.sbuf_tensor("xtall", [P, NRT * 2 * F], u16))

        def xsl(rt, a, b):
            return xtall[:, rt * 2 * F + a:rt * 2 * F + b]
        poall = ctx.enter_context(
            nc.sbuf_tensor("poall", [P, NRT * NPAIR * F], u16))
        po = [poall[:, rt * NPAIR * F:(rt + 1) * NPAIR * F]
              for rt in range(NRT)]

        in_sem = [ctx.enter_context(nc.semaphore(f"in_sem{b}"))
                  for b in range(2)]
        in0b_sem = ctx.enter_context(nc.semaphore("in0b_sem"))
        in23_sem = ctx.enter_context(nc.semaphore("in23_sem")) \
            if gp_bulk else None
        vd_sem = ctx.enter_context(nc.semaphore("vd_sem"))
        od_sem = ctx.enter_context(nc.semaphore("od_sem"))

        ctx.enter_context(nc.Block())
        block = nc.cur_block

        def emit_piece(eng, rt, q0, q1):
            if rt == "b23":        # p-half of the merged rt2+rt3 region
                p0, p1 = q0, q1
                eng.wait_ge(vd_sem, 7)
                eng.dma_start(
                    out_b23[p0:p1, :, :, :],
                    poall[p0:p1, 2 * NPAIR * F:4 * NPAIR * F]
                    .rearrange("p (t q f) -> p t q f", t=2, q=NPAIR),
                ).then_inc(od_sem, 16)
                return
            # sparse milestone counts: rt0 planes 1/2/4/8/16 -> 1..5, rt1 -> 6
            if rt == 0:
                eng.wait_ge(vd_sem, {1: 1, 2: 2, 4: 3, 8: 4, 16: 5}[q1])
            else:
                eng.wait_ge(vd_sem, 5 + rt)
            eng.dma_start(
                out_rqf[rt * P:(rt + 1) * P, q0:q1, :],
                po[rt][:, q0 * F:q1 * F]
                .rearrange("p (q f) -> p q f", f=F),
            ).then_inc(od_sem, 16)

        # vd_sem increments only at the milestones pieces actually wait on
        # (sparse sem traffic keeps the DVE pipeline tighter): rt0 planes
        # 1/2/4/8/16 done -> counts 1..5, rt1 done -> 6, rt2+rt3 done -> 7.
        RT0_MARKS = {0: 1, 1: 2, 3: 3, 7: 4, 15: 5}

        @block.vector
        def _(vec: bass.BassEngine):
            for rt in range(2 if pair23 else NRT):
                for q in range(NPAIR):
                    if rt == 0 and q == 0:
                        vec.wait_ge(in_sem[0], 32)       # rt0 hi plane halves
                    elif rt == 0 and q == 8:
                        vec.wait_ge(in0b_sem, 32)        # rt0 lo plane halves
                    elif rt == 1 and q == 0:
                        vec.wait_ge(in_sem[1], 16)       # rt1(-3) bulk arrived
                    elif gp_bulk and rt == 2 and q == 0:
                        vec.wait_ge(in23_sem, 16)        # rt2-3 via gpsimd
                    plane, s = (0, q) if q < 8 else (F, q - 8)
                    o = po[rt][:, q * F:(q + 1) * F]
                    i0 = xsl(rt, plane, plane + F)
                    inst = vec.tensor_scalar(o, i0, s, 0x0101, SHR, AND)
                    if rt == 0 and q in RT0_MARKS:
                        inst.then_inc(vd_sem)
                    elif rt >= 1 and q == NPAIR - 1:
                        inst.then_inc(vd_sem)
            if pair23:
                # one 1024-elem instruction per plane covering rt2 AND rt3:
                # both tiles' bytes become piece-ready ~4us earlier with no
                # descriptor-size penalty (still 4x_2p: last dim packed u16)
                o23 = poall[:, 2 * NPAIR * F:4 * NPAIR * F] \
                    .rearrange("p (r x) -> p r x", r=2)
                i23 = xtall[:, 4 * F:NRT * 2 * F] \
                    .rearrange("p (r x) -> p r x", r=2)
                for q in range(NPAIR):
                    if q == 0:
                        if gp_bulk:
                            vec.wait_ge(in23_sem, 16)
                    plane, s = (0, q) if q < 8 else (F, q - 8)
                    o = o23[:, :, q * F:(q + 1) * F]
                    i0 = i23[:, :, plane:plane + F]
                    inst = vec.tensor_scalar(o, i0, s, 0x0101, SHR, AND)
                    if q == NPAIR - 1:
                        inst.then_inc(vd_sem)

        @block.scalar
        def _(sc: bass.BassEngine):
            # rt0 input split by PARTITION across the sync/scalar queues:
            # same two-queue parallelism as an f-split but with 1-2KB
            # descriptors instead of 512B (17.9 -> 21.6 B/ns)
            sc.dma_start(
                xtall[P // 2:P, 0:F], xm_ap[P // 2:P, 0:F]
            ).then_inc(in_sem[0], 16)
            sc.dma_start(
                xtall[P // 2:P, F:2 * F], xm_ap[P // 2:P, F:2 * F]
            ).then_inc(in0b_sem, 16)
            # rt1(-3) input bulk: large descriptors
            bulk_end = 4 * F if gp_bulk else NRT * 2 * F
            sc.dma_start(
                xtall[:, 2 * F:bulk_end], xm_ap[0:P, 2 * F:bulk_end]
            ).then_inc(in_sem[1], 16)
            if tensor_q:
                for i, (rt, q0, q1) in enumerate(pieces):
                    if i % 2 == 1:
                        emit_piece(sc, rt, q0, q1)

        if gp_bulk:
            # rt2-3 input rides the slow gpsimd SWDGE queue: its ~3us
            # issue-to-land latency drops these bytes into the engine-idle
            # hole between the in-stream drain and the first out piece.
            @block.gpsimd
            def _(gp: bass.BassEngine):
                gp.dma_start(
                    xtall[:, 4 * F:NRT * 2 * F],
                    xm_ap[0:P, 4 * F:NRT * 2 * F]
                ).then_inc(in23_sem, 16)

        @block.sync
        def _(sp: bass.BassEngine):
            # rt0 input: hi plane first so VectorE starts after 128KB
            sp.dma_start(
                xtall[0:P // 2, 0:F], xm_ap[0:P // 2, 0:F]
            ).then_inc(in_sem[0], 16)
            sp.dma_start(
                xtall[0:P // 2, F:2 * F], xm_ap[0:P // 2, F:2 * F]
            ).then_inc(in0b_sem, 16)
            for i, (rt, q0, q1) in enumerate(pieces):
                if tensor_q and i % 2 == 1:
                    continue
                emit_piece(sp, rt, q0, q1)

    return nc


_NC_CACHE = None


def _get_nc():
    global _NC_CACHE
    if _NC_CACHE is None:
        _NC_CACHE = build_nc()
    return _NC_CACHE


def pack_shard(x_shard: np.ndarray) -> np.ndarray:
    """[ROWS, F] f32 -> [P, NRT*2F] u16, partition-major: SBUF row p holds
    [rt0 hi|lo][rt1 hi|lo][rt2][rt3] for source rows rt*128+p, where hi is
    bits 31..16 (bit 31 replaced by the x<0 test) and lo is bits 15..0."""
    x_shard = np.ascontiguousarray(x_shard)
    xi = x_shard.view(np.uint32)
    xi = (xi & np.uint32(0x7FFFFFFF)) | \
        ((x_shard < 0).astype(np.uint32) << np.uint32(31))
    hi = (xi >> np.uint32(16)).astype(np.uint16)     # [ROWS, F]
    lo = (xi & np.uint32(0xFFFF)).astype(np.uint16)
    # [ROWS, 2, F] -> [NRT, P, 2, F] -> [P, NRT, 2, F] -> [P, NRT*2F]
    planes = np.stack([hi, lo], axis=1).reshape(NRT, P, 2, F)
    return np.ascontiguousarray(
        planes.transpose(1, 0, 2, 3).reshape(P, NRT * 2 * F))


def unpack_result(out_dev: np.ndarray, big23: bool = False,
                  b23_16k: bool = False) -> np.ndarray:
    """Device [ROWS*NPAIR, F] u16 -> [ROWS, F, K] f32 in channel order."""
    rqf = out_dev.reshape(ROWS, NPAIR, F)
    if big23 and not b23_16k:
        # rows 256..511 are stored partition-major [p, rt-2, q, f]
        head = rqf[:2 * P]
        tail = out_dev[2 * P * NPAIR:].reshape(P, 2, NPAIR, F) \
            .transpose(1, 0, 2, 3).reshape(2 * P, NPAIR, F)
        rqf = np.concatenate([head, tail], axis=0)
    # b23_16k keeps [t, p, q, f] = natural row order: no reshuffle needed
    raw = np.ascontiguousarray(rqf).view(np.uint8).reshape(ROWS, NPAIR, F, 2)
    byte_k = raw.transpose(0, 2, 1, 3).reshape(ROWS, F, K)
    return byte_k[:, :, PERM].astype(np.float32)


def kernel(x: np.ndarray) -> np.ndarray:
    from concourse.bass_utils import run_bass_kernel_spmd

    x = np.asarray(x, dtype=np.float32)
    assert x.shape == (ROWS_TOTAL, F), x.shape
    nc = _get_nc()
    in_maps = [
        {"xm": pack_shard(x[i * ROWS:(i + 1) * ROWS])} for i in range(N_CORES)
    ]
    res = run_bass_kernel_spmd(nc, in_maps, list(range(N_CORES)))
    parts = [unpack_result(res.results[i]["out"], big23=True, b23_16k=True)
             for i in range(N_CORES)]
    return np.concatenate(parts, axis=0)
